# revision 1
# baseline (speedup 1.0000x reference)
"""Multi-head causal attention (RoPE, muP scale) on 8 TRN2 NeuronCores.

Sharding: core c = (b, g) with b = c // 4 (data-parallel batch), g = c % 4
(tensor-parallel head group of 4 heads).  Each core computes q/k/v
projections for its 4 heads, RoPE, causal flash-style attention in the
transposed (sT = [k, q]) orientation, and a partial output projection
o @ wo over its heads.  The host sums the 4 per-group partials per batch
(the tensor-parallel reduce) and stacks the 2 batches.

Matmuls run as float32r (FP22 reads, 1.5 cyc/row) accumulating in fp32
PSUM; softmax runs without max-subtraction (scores are O(0.1) by muP
scaling so exp is well-conditioned), with the causal mask applied as a
0/1 multiply on the diagonal blocks and the denominator accumulated by
an all-ones matmul into a broadcast PSUM tile.
"""

import sys

if "/opt/trn_rl_repo" not in sys.path:
    sys.path.insert(0, "/opt/trn_rl_repo")

import numpy as np

B, T, M, H, D = 2, 2048, 2048, 16, 128
N_CORES = 8
GROUPS = 4
HPG = H // GROUPS          # heads per group (4)
ROTARY_BASE = 10000.0
ATTN_SCALE = 1.0 / 128.0

P = 128                    # partitions
TC = T // 512              # 4 t-chunks of 512
MB = M // P                # 16 m-blocks
TB = T // P                # 16 t-blocks
NQ = 512                   # q-chunk width

_CACHE = {}


def _build_program():
    from concourse import bacc, tile
    import concourse.mybir as mybir

    F32 = mybir.dt.float32
    F32R = mybir.dt.float32r
    AFT = mybir.ActivationFunctionType

    nc = bacc.Bacc("TRN2", target_bir_lowering=False, debug=False,
                   num_devices=N_CORES)

    xt_d = nc.dram_tensor("xt", [M, T], F32R, kind="ExternalInput")
    wq_d = nc.dram_tensor("wq", [P, 2, MB, 256], F32R, kind="ExternalInput")
    wk_d = nc.dram_tensor("wk", [P, 2, MB, 256], F32R, kind="ExternalInput")
    wv_d = nc.dram_tensor("wv", [P, 2, MB, 256], F32R, kind="ExternalInput")
    wo_d = nc.dram_tensor("wo", [P, HPG, M], F32R, kind="ExternalInput")
    cc_d = nc.dram_tensor("trig_cc", [P, T], F32, kind="ExternalInput")
    ss_d = nc.dram_tensor("trig_ss", [P, T], F32, kind="ExternalInput")
    mask_d = nc.dram_tensor("mask01", [P, 4 * NQ], F32, kind="ExternalInput")
    ones_d = nc.dram_tensor("onesw", [P, P], F32R, kind="ExternalInput")
    r_d = nc.dram_tensor("r_out", [T, M], F32, kind="ExternalOutput")

    with tile.TileContext(nc) as tc:
        with (
            tc.tile_pool(name="consts", bufs=1) as consts,
            tc.tile_pool(name="wpool", bufs=1) as wpool,
            tc.tile_pool(name="wopool", bufs=2) as wopool,
            tc.tile_pool(name="qkv", bufs=1) as qkv,
            tc.tile_pool(name="xpool", bufs=6) as xpool,
            tc.tile_pool(name="ppool", bufs=2) as ppool,
            tc.tile_pool(name="rtmp", bufs=2) as rtmp,
            tc.tile_pool(name="opool", bufs=1) as opool,
            tc.tile_pool(name="rout", bufs=2) as rout,
            tc.tile_pool(name="psum", bufs=1, space="PSUM") as psum,
        ):
            cc_sb = consts.tile([P, T], F32, tag="cc")
            nc.sync.dma_start(out=cc_sb[:], in_=cc_d[:])
            ss_sb = consts.tile([P, T], F32, tag="ss")
            nc.sync.dma_start(out=ss_sb[:], in_=ss_d[:])
            mask_sb = consts.tile([P, 4 * NQ], F32, tag="mask")
            nc.sync.dma_start(out=mask_sb[:], in_=mask_d[:])
            ones_sb = consts.tile([P, P], F32R, tag="ones")
            nc.sync.dma_start(out=ones_sb[:], in_=ones_d[:])

            # oT for all 4 heads of the group: [d, h4 * T + t]
            oT_sb = opool.tile([P, HPG * T], F32R, tag="oT")

            xt_tiles = {}

            for pair in range(2):
                wq_sb = wpool.tile([P, MB, 256], F32R, tag="wq", name="wq_sb")
                nc.sync.dma_start(out=wq_sb[:], in_=wq_d[:, pair])
                wk_sb = wpool.tile([P, MB, 256], F32R, tag="wk", name="wk_sb")
                nc.sync.dma_start(out=wk_sb[:], in_=wk_d[:, pair])
                wv_sb = wpool.tile([P, MB, 256], F32R, tag="wv", name="wv_sb")
                nc.sync.dma_start(out=wv_sb[:], in_=wv_d[:, pair])

                qT_sb = [qkv.tile([P, T], F32R, tag=f"qT{hl}", name=f"qT{hl}")
                         for hl in range(2)]
                kT_sb = [qkv.tile([P, T], F32R, tag=f"kT{hl}", name=f"kT{hl}")
                         for hl in range(2)]
                v_sb = qkv.tile([P, TB * 256], F32R, tag="v", name="v_sb")

                # ---- projections + RoPE for this head pair ----
                # xt tile prefetcher: explicit round-robin tags (FIFO slot
                # reuse) and next-chunk DMAs emitted inside the current
                # chunk's tail so the scheduler places them (and their slot
                # waits) early enough to hide the refill at chunk boundaries.
                def ensure_xt(i):
                    if i in xt_tiles:
                        return xt_tiles[i]
                    tcx_i = (i // MB) % TC
                    mb_i = i % MB
                    xt_t = xpool.tile([P, NQ], F32R, bufs=1,
                                      tag=f"xt{i % 8}", name="xt_t")
                    eng = nc.sync if i % 2 == 0 else nc.gpsimd
                    eng.dma_start(
                        out=xt_t[:],
                        in_=xt_d[mb_i * P:(mb_i + 1) * P,
                                 tcx_i * NQ:(tcx_i + 1) * NQ],
                    )
                    xt_tiles[i] = xt_t
                    return xt_t

                for tcx in range(TC):
                    gidx = pair * TC + tcx
                    tsl = slice(tcx * NQ, (tcx + 1) * NQ)
                    psq = [psum.tile([P, NQ], F32, tag=f"q{hl}", name=f"psq{hl}")
                           for hl in range(2)]
                    psk = [psum.tile([P, NQ], F32, tag=f"k{hl}", name=f"psk{hl}")
                           for hl in range(2)]
                    psv = [psum.tile([P, 256], F32, tag=f"v{ts}", name=f"psv{ts}")
                           for ts in range(4)]
                    for mb in range(MB):
                        i = gidx * MB + mb
                        xt_t = ensure_xt(i)
                        xt_tiles.pop(i, None)
                        if mb >= MB - 8 and i + 8 < 2 * TC * MB:
                            ensure_xt(i + 8)
                        st, sp = (mb == 0), (mb == MB - 1)
                        # v first: its PSUM banks free fastest at chunk
                        # boundaries, keeping PE fed while RoPE drains q/k.
                        for ts in range(4):
                            nc.tensor.matmul(
                                psv[ts][:], xt_t[:, ts * P:(ts + 1) * P],
                                wv_sb[:, mb, :], start=st, stop=sp)
                        for hl in range(2):
                            nc.tensor.matmul(
                                psq[hl][:], wq_sb[:, mb, hl * P:(hl + 1) * P],
                                xt_t[:], start=st, stop=sp)
                            nc.tensor.matmul(
                                psk[hl][:], wk_sb[:, mb, hl * P:(hl + 1) * P],
                                xt_t[:], start=st, stop=sp)

                    for ts in range(4):
                        tb = tcx * 4 + ts
                        nc.scalar.activation(
                            v_sb[:, tb * 256:(tb + 1) * 256], psv[ts][:], AFT.Copy)

                    # RoPE: rot_even = qe*cos - qo*sin ; rot_odd = qe*sin + qo*cos
                    # Phase 1 per psum bank: one full-width DVE product against
                    # the duplicated-cos tile (pab = [qe*cos ; qo*cos]) plus a
                    # partition-half swap done as two cross-base ScalarE copies
                    # off the PSUM operand (sh = [qo ; qe]).  This releases the
                    # projection PSUM bank quickly so the next chunk's matmuls
                    # start immediately.  Phase 2 (off the critical path):
                    # pcd = sh * sin_dup = [qo*sin ; qe*sin], then same-base
                    # DVE combines.
                    groups = []
                    for hl in range(2):
                        groups.append((psq[hl], qT_sb[hl]))
                        groups.append((psk[hl], kT_sb[hl]))
                    phase2 = []
                    for ps, dst in groups:
                        pab = rtmp.tile([P, NQ], F32, tag="pab", name="pab",
                                        bufs=3)
                        nc.vector.tensor_mul(pab[:], ps[:], cc_sb[:, tsl])
                        sh = rtmp.tile([P, NQ], F32, tag="sh", name="sh",
                                       bufs=2)
                        # partition-half swap via cross-base copies off a PSUM
                        # operand (ScalarE, off the DVE critical path)
                        nc.scalar.activation(sh[0:64, :], ps[64:128, :], AFT.Copy)
                        nc.scalar.activation(sh[64:128, :], ps[0:64, :], AFT.Copy)
                        phase2.append((pab, sh, dst))
                    for pab, sh, dst in phase2:
                        pcd = rtmp.tile([P, NQ], F32, tag="pcd", name="pcd", bufs=1)
                        nc.vector.tensor_mul(pcd[:], sh[:], ss_sb[:, tsl])
                        nc.vector.tensor_sub(
                            dst[0:64, tsl], pab[0:64, :], pcd[0:64, :])
                        nc.vector.tensor_add(
                            dst[64:128, tsl], pcd[64:128, :], pab[64:128, :])


                # ---- attention for the two heads of this pair ----
                for hl in range(2):
                    h4 = pair * 2 + hl
                    for qc in range(TC):
                        ps_oT = psum.tile([P, NQ], F32,
                                          tag=("v0", "k0")[qc % 2], name="ps_oT")
                        ps_den = psum.tile([P, NQ], F32,
                                           tag=("v1", "k1")[qc % 2], name="ps_den")
                        jmax = 4 * qc + 3
                        for j in range(jmax + 1):
                            pat = j - 4 * qc
                            q0 = 128 * pat if pat >= 0 else 0
                            qs = slice(qc * NQ + q0, (qc + 1) * NQ)
                            st, sp = (j == 0), (j == jmax)
                            ps_sT = psum.tile([P, NQ], F32,
                                              tag=("q0", "q1", "v2")[j % 3],
                                              name="ps_sT")
                            nc.tensor.matmul(
                                ps_sT[:, q0:NQ],
                                kT_sb[hl][:, j * P:(j + 1) * P],
                                qT_sb[hl][:, qs], start=True, stop=True)
                            pT = ppool.tile([P, NQ], F32R, tag="pT", name="pT")
                            nc.scalar.activation(
                                pT[:, q0:NQ], ps_sT[:, q0:NQ], AFT.Exp)
                            if pat >= 0:
                                nc.vector.tensor_mul(
                                    pT[:, q0:NQ],
                                    pT[:, q0:NQ].bitcast(F32),
                                    mask_sb[:, pat * NQ + q0:(pat + 1) * NQ])
                            nc.tensor.matmul(
                                ps_oT[:, q0:NQ],
                                v_sb[:, j * 256 + hl * P: j * 256 + hl * P + P],
                                pT[:, q0:NQ], start=st, stop=sp)
                            nc.tensor.matmul(
                                ps_den[:, q0:NQ], ones_sb[:],
                                pT[:, q0:NQ], start=st, stop=sp)
                        rec = rtmp.tile([P, NQ], F32, tag="rec", name="rec", bufs=1)
                        nc.vector.reciprocal(rec[:], ps_den[:])
                        nc.vector.tensor_mul(
                            oT_sb[:, h4 * T + qc * NQ: h4 * T + (qc + 1) * NQ],
                            ps_oT[:], rec[:])

            # ---- output projection: r[t, m] = sum_h oT_h.T @ wo_h ----
            for mc in range(4):
                wo_mc = wopool.tile([P, HPG, NQ], F32R, tag="womc", name="wo_mc")
                nc.sync.dma_start(out=wo_mc[:], in_=wo_d[:, :, mc * NQ:(mc + 1) * NQ])
                for tb in range(TB):
                    ps_r = psum.tile([P, NQ], F32, tag=("q0", "q1", "v2")[tb % 3], name="ps_r")
                    for h4 in range(HPG):
                        nc.tensor.matmul(
                            ps_r[:],
                            oT_sb[:, h4 * T + tb * P: h4 * T + (tb + 1) * P],
                            wo_mc[:, h4, :],
                            start=(h4 == 0), stop=(h4 == HPG - 1))
                    ro = rout.tile([P, NQ], F32, tag="ro", name="ro")
                    nc.scalar.activation(ro[:], ps_r[:], AFT.Copy)
                    # store on the gpsimd queue so wo prefetch (sync queue)
                    # is not blocked behind 16 result stores
                    nc.gpsimd.dma_start(
                        out=r_d[tb * P:(tb + 1) * P, mc * NQ:(mc + 1) * NQ],
                        in_=ro[:])

    nc.compile()
    return nc


def _host_constants():
    half = D // 2
    pos = np.arange(T, dtype=np.float32)[:, None]
    freqs = np.power(
        np.float32(ROTARY_BASE),
        -np.arange(half, dtype=np.float32) / np.float32(half))[None, :]
    rad = pos * freqs                              # [T, 64]
    cos = np.cos(rad).astype(np.float32).T         # [64, T]
    sin = np.sin(rad).astype(np.float32).T         # [64, T]
    cc = np.concatenate([cos, cos], axis=0)        # cos duplicated on both halves
    ss = np.concatenate([sin, sin], axis=0)        # sin duplicated

    # mask pattern p: allowed (1.0) where 128*p + kk <= qq
    kk = np.arange(P)[:, None]
    qq = np.arange(NQ)[None, :]
    mask = np.concatenate(
        [(P * p + kk <= qq).astype(np.float32) for p in range(4)], axis=1)

    ones = np.ones((P, P), dtype=np.float32)
    return cc, ss, mask, ones


def kernel(x, wq, wk, wv, wo):
    x = np.ascontiguousarray(np.asarray(x, dtype=np.float32))
    wq = np.asarray(wq, dtype=np.float32)
    wk = np.asarray(wk, dtype=np.float32)
    wv = np.asarray(wv, dtype=np.float32)
    wo = np.asarray(wo, dtype=np.float32)

    from concourse.bass_utils import run_bass_kernel_spmd

    if "nc" not in _CACHE:
        _CACHE["nc"] = _build_program()
    nc = _CACHE["nc"]

    cc, ss, mask, ones = _host_constants()
    mult = np.float32(np.sqrt(ATTN_SCALE))

    def w_layout(w, g, scale):
        # w: [M, H, D] -> group slice -> [P, 2, MB, 256]
        ws = (w[:, g * HPG:(g + 1) * HPG, :] * scale).astype(np.float32)
        ws = ws.reshape(M, 2, 256)                    # pair-major head axis
        ws = ws.reshape(MB, P, 2, 256).transpose(1, 2, 0, 3)
        return np.ascontiguousarray(ws)

    in_maps = []
    for c in range(N_CORES):
        b, g = divmod(c, GROUPS)
        xt = np.ascontiguousarray(x[b].T)            # [M, T]
        wog = np.ascontiguousarray(
            wo[g * HPG:(g + 1) * HPG].transpose(1, 0, 2))  # [D, HPG, M]
        in_maps.append({
            "xt": xt,
            "wq": w_layout(wq, g, mult),
            "wk": w_layout(wk, g, mult),
            "wv": w_layout(wv, g, np.float32(1.0)),
            "wo": wog,
            "trig_cc": cc,
            "trig_ss": ss,
            "mask01": mask,
            "onesw": ones,
        })

    res = run_bass_kernel_spmd(nc, in_maps, list(range(N_CORES)))

    r = np.zeros((B, T, M), dtype=np.float32)
    for c in range(N_CORES):
        b = c // GROUPS
        r[b] += res.results[c]["r_out"]
    return r



# revision 7
# speedup vs baseline: 1.4753x; 1.4753x over previous
"""Multi-head causal attention (RoPE, muP scale) on 8 TRN2 NeuronCores.

Sharding: core c = (b, g) with b = c // 4 (data-parallel batch), g = c % 4
(tensor-parallel head group of 4 heads).  Each core computes q/k/v
projections for its 4 heads, RoPE, causal flash-style attention in the
transposed (sT = [k, q]) orientation, and a partial output projection
o @ wo over its heads.  The host sums the 4 per-group partials per batch
(the tensor-parallel reduce) and stacks the 2 batches.

All matmul operands are bf16 (PSUM accumulation stays fp32): x is held
resident in SBUF (8 MB) and loaded once via per-mb-block DMAs so the
first projection matmul issues ~2us in.  The causal mask is applied on
the tensor engine as an identity-matmul accumulate of a -1e30 upper
triangle into the diagonal score blocks before exp; softmax denominators
come from an all-ones matmul.  RoPE uses a tan formulation so a single
DVE multiply retires each projection PSUM bank.  PSUM banks are assigned
explicitly (b0..b7: q/k accumulation on b0..b3, v and the attention
score/output-projection rotation on b4..b7); attention interleaves both
heads per j-block, runs pv/den two blocks behind exp so the PE never
waits on ACT, and drains oT/den to SBUF with cheap DVE copies so the
(slow) reciprocal runs entirely off the bank-reuse critical path.
"""

import sys

if "/opt/trn_rl_repo" not in sys.path:
    sys.path.insert(0, "/opt/trn_rl_repo")

import numpy as np

B, T, M, H, D = 2, 2048, 2048, 16, 128
N_CORES = 8
GROUPS = 4
HPG = H // GROUPS          # heads per group (4)
ROTARY_BASE = 10000.0
ATTN_SCALE = 1.0 / 128.0

P = 128                    # partitions
TC = T // 512              # 4 t-chunks of 512
MB = M // P                # 16 m-blocks
TB = T // P                # 16 t-blocks
NQ = 512                   # q-chunk width

_CACHE = {}


def _build_program():
    from concourse import bacc, tile
    import concourse.mybir as mybir

    F32 = mybir.dt.float32
    BF16 = mybir.dt.bfloat16
    AFT = mybir.ActivationFunctionType

    nc = bacc.Bacc("TRN2", target_bir_lowering=False, debug=False,
                   num_devices=N_CORES)

    xt_d = nc.dram_tensor("xt", [M, T], BF16, kind="ExternalInput")
    wq_d = nc.dram_tensor("wq", [P, 2, MB, 256], BF16, kind="ExternalInput")
    wk_d = nc.dram_tensor("wk", [P, 2, MB, 256], BF16, kind="ExternalInput")
    wv_d = nc.dram_tensor("wv", [P, 2, MB, 256], BF16, kind="ExternalInput")
    wo_d = nc.dram_tensor("wo", [P, HPG, M], BF16, kind="ExternalInput")
    cc_d = nc.dram_tensor("trig_cc", [P, T], BF16, kind="ExternalInput")
    tt_d = nc.dram_tensor("trig_tt", [P, T], BF16, kind="ExternalInput")
    tri_d = nc.dram_tensor("tri_neg", [P, P], BF16, kind="ExternalInput")
    ones_d = nc.dram_tensor("onesw", [P, P], BF16, kind="ExternalInput")
    id_d = nc.dram_tensor("identw", [P, P], BF16, kind="ExternalInput")
    r_d = nc.dram_tensor("r_out", [T, M], BF16, kind="ExternalOutput")

    with tile.TileContext(nc) as tc:
        with (
            tc.tile_pool(name="consts", bufs=1) as consts,
            tc.tile_pool(name="xpool", bufs=1) as xpool,
            tc.tile_pool(name="wpool", bufs=2) as wpool,
            tc.tile_pool(name="wopool", bufs=2) as wopool,
            tc.tile_pool(name="qkv", bufs=1) as qkv,
            tc.tile_pool(name="ppool", bufs=6) as ppool,
            tc.tile_pool(name="rtmp", bufs=2) as rtmp,
            tc.tile_pool(name="opool", bufs=1) as opool,
            tc.tile_pool(name="rout", bufs=3) as rout,
            tc.tile_pool(name="psum", bufs=1, space="PSUM") as psum,
        ):
            # --- load queues: alternate the two cheap DMA triggers ---
            qs = [nc.sync, nc.gpsimd]
            qi = [0]

            def ld(out, in_):
                qs[qi[0] % 2].dma_start(out=out, in_=in_)
                qi[0] += 1

            tri_sb = consts.tile([P, P], BF16, tag="tri")
            ld(tri_sb[:], tri_d[:])
            ones_sb = consts.tile([P, P], BF16, tag="ones")
            ld(ones_sb[:], ones_d[:])
            id_sb = consts.tile([P, P], BF16, tag="ident")
            ld(id_sb[:], id_d[:])

            # resident x^T [m, t], one DMA per 128-row m-block, interleaved
            # with the pair-0 weight quarters in consumption order
            xt_sb = xpool.tile([P, MB, T], BF16, tag="xt")
            w_sb = {}   # (pair) -> (wq, wk, wv) tiles
            for pair in range(2):
                w_sb[pair] = (
                    wpool.tile([P, MB, 256], BF16, tag="wq", name="wq_sb"),
                    wpool.tile([P, MB, 256], BF16, tag="wk", name="wk_sb"),
                    wpool.tile([P, MB, 256], BF16, tag="wv", name="wv_sb"),
                )

            def load_pair_weights(pair):
                wq_t, wk_t, wv_t = w_sb[pair]
                for q4 in range(4):
                    sl = slice(4 * q4, 4 * q4 + 4)
                    ld(wv_t[:, sl, :], wv_d[:, pair, sl, :])
                    ld(wq_t[:, sl, :], wq_d[:, pair, sl, :])
                    ld(wk_t[:, sl, :], wk_d[:, pair, sl, :])
                    if pair == 0:
                        for mb in range(4 * q4, 4 * q4 + 4):
                            ld(xt_sb[:, mb, :], xt_d[mb * P:(mb + 1) * P, :])

            load_pair_weights(0)
            cc_sb = consts.tile([P, T], BF16, tag="cc")
            ld(cc_sb[:], cc_d[:])
            tt_sb = consts.tile([P, T], BF16, tag="tt")
            ld(tt_sb[:], tt_d[:])

            # oT for all 4 heads of the group: [d, h4 * T + t], bf16
            oT_sb = opool.tile([P, HPG * T], BF16, tag="oT")
            pend_norm = []   # deferred (h4, qc, denc, oc)

            for pair in range(2):
                wq_t, wk_t, wv_t = w_sb[pair]
                if pair == 1:
                    load_pair_weights(1)

                qT_sb = [qkv.tile([P, T], BF16, tag=f"qT{hl}", name=f"qT{hl}")
                         for hl in range(2)]
                kT_sb = [qkv.tile([P, T], BF16, tag=f"kT{hl}", name=f"kT{hl}")
                         for hl in range(2)]
                v_sb = qkv.tile([P, TB * 256], BF16, tag="v", name="v_sb")

                # ---- projections + RoPE, one 512-wide t-chunk at a time ----
                for tcx in range(TC):
                    tsl = slice(tcx * NQ, (tcx + 1) * NQ)
                    psq = [psum.tile([P, NQ], F32, tag=("b0", "b1")[hl],
                                     name=f"psq{hl}") for hl in range(2)]
                    psk = [psum.tile([P, NQ], F32, tag=("b2", "b3")[hl],
                                     name=f"psk{hl}") for hl in range(2)]
                    # one full PSUM bank per v accumulation group (only the
                    # first 256 columns are written — a bank holds a single
                    # accumulation group)
                    psv = [psum.tile([P, NQ], F32, tag=f"b{4 + ts}",
                                     name=f"psv{ts}") for ts in range(4)]

                    # emit v matmuls two m-blocks ahead of q/k so the chunk
                    # opens with work whose banks freed earliest
                    jobs = []
                    for mb in range(MB):
                        jobs.append(("v", mb))
                        if mb >= 2:
                            jobs.append(("qk", mb - 2))
                    jobs += [("qk", MB - 2), ("qk", MB - 1)]

                    for kind, mb in jobs:
                        st, sp = (mb == 0), (mb == MB - 1)
                        if kind == "v":
                            for ts in range(4):
                                nc.tensor.matmul(
                                    psv[ts][:, 0:256],
                                    xt_sb[:, mb, tcx * NQ + ts * P:
                                          tcx * NQ + (ts + 1) * P],
                                    wv_t[:, mb, :], start=st, stop=sp)
                        else:
                            for hl in range(2):
                                nc.tensor.matmul(
                                    psq[hl][:],
                                    wq_t[:, mb, hl * P:(hl + 1) * P],
                                    xt_sb[:, mb, tsl], start=st, stop=sp)
                                nc.tensor.matmul(
                                    psk[hl][:],
                                    wk_t[:, mb, hl * P:(hl + 1) * P],
                                    xt_sb[:, mb, tsl], start=st, stop=sp)

                    # v bank drains (ACT) — free b4..b7 for the next chunk
                    for ts in range(4):
                        tb = tcx * 4 + ts
                        nc.scalar.activation(
                            v_sb[:, tb * 256:(tb + 1) * 256],
                            psv[ts][:, 0:256], AFT.Copy)

                    # RoPE.  rot_even = qe*cos - qo*sin ; rot_odd = qe*sin +
                    # qo*cos.  pab = [qe*cos ; qo*cos] in one DVE op against
                    # the duplicated-cos table — the only reader of the
                    # projection PSUM bank (emitted q0,k0,q1,k1 to match the
                    # next chunk's bank-need order).  sh = swap(pab) (ACT),
                    # then the sin products are sh * tan.
                    phase2 = []
                    for ps, dst, nm in (
                        (psq[0], qT_sb[0], "q0"), (psk[0], kT_sb[0], "k0"),
                        (psq[1], qT_sb[1], "q1"), (psk[1], kT_sb[1], "k1"),
                    ):
                        pab = rtmp.tile([P, NQ], F32, tag="pab", name="pab",
                                        bufs=4)
                        nc.vector.tensor_mul(pab[:], ps[:], cc_sb[:, tsl])
                        phase2.append((pab, dst))
                    for pab, dst in phase2:
                        sh = rtmp.tile([P, NQ], BF16, tag="sh", name="sh",
                                       bufs=2)
                        nc.scalar.activation(sh[0:64, :], pab[64:128, :],
                                             AFT.Copy)
                        nc.scalar.activation(sh[64:128, :], pab[0:64, :],
                                             AFT.Copy)
                        pcd = rtmp.tile([P, NQ], BF16, tag="pcd", name="pcd",
                                        bufs=2)
                        nc.vector.tensor_mul(pcd[:], sh[:], tt_sb[:, tsl])
                        nc.vector.tensor_sub(
                            dst[0:64, tsl], pab[0:64, :], pcd[0:64, :])
                        nc.vector.tensor_add(
                            dst[64:128, tsl], pcd[64:128, :], pab[64:128, :])

                # ---- attention: heads interleaved per j-block, pv/den two
                # blocks behind exp, oT/den drained to SBUF so the slow
                # reciprocal runs off the bank critical path ----
                sT_tags = ("b4", "b5", "b6", "b7")
                sidx = 0
                for qc in range(TC):
                    ps_oT = [psum.tile([P, NQ], F32, tag=("b0", "b2")[hl],
                                       name=f"ps_oT{hl}") for hl in range(2)]
                    ps_den = [psum.tile([P, NQ], F32, tag=("b1", "b3")[hl],
                                        name=f"ps_den{hl}")
                              for hl in range(2)]
                    jmax = 4 * qc + 3
                    pend_pv = []
                    for j in range(jmax + 1):
                        pat = j - 4 * qc
                        q0 = 128 * pat if pat >= 0 else 0
                        qs_ = slice(qc * NQ + q0, (qc + 1) * NQ)
                        st, sp = (j == 0), (j == jmax)
                        for hl in range(2):
                            ps_sT = psum.tile([P, NQ], F32,
                                              tag=sT_tags[sidx % 4],
                                              name="ps_sT")
                            sidx += 1
                            if pat >= 0:
                                # diagonal block: add the -1e30 triangle on
                                # the tensor engine before exp
                                nc.tensor.matmul(
                                    ps_sT[:, q0:NQ],
                                    kT_sb[hl][:, j * P:(j + 1) * P],
                                    qT_sb[hl][:, qs_], start=True, stop=False)
                                nc.tensor.matmul(
                                    ps_sT[:, q0:q0 + 128], id_sb[:],
                                    tri_sb[:], start=False, stop=True)
                            else:
                                nc.tensor.matmul(
                                    ps_sT[:, q0:NQ],
                                    kT_sb[hl][:, j * P:(j + 1) * P],
                                    qT_sb[hl][:, qs_], start=True, stop=True)
                            pT = ppool.tile([P, NQ], BF16, tag="pT", name="pT")
                            nc.scalar.activation(
                                pT[:, q0:NQ], ps_sT[:, q0:NQ], AFT.Exp)
                            pend_pv.append((hl, j, q0, pT, st, sp))
                            if len(pend_pv) > 4:
                                _emit_pv(nc, pend_pv.pop(0), ps_oT, ps_den,
                                         v_sb, ones_sb)
                    while pend_pv:
                        _emit_pv(nc, pend_pv.pop(0), ps_oT, ps_den, v_sb,
                                 ones_sb)
                    # drain copies (cheap, frees banks for qc+1) ...
                    for hl in range(2):
                        denc = rtmp.tile([P, NQ], F32, tag="denc",
                                         name="denc", bufs=3)
                        nc.vector.tensor_scalar_add(denc[:], ps_den[hl][:],
                                                    0.0)
                        oc = rtmp.tile([P, NQ], F32, tag="oc", name="oc",
                                       bufs=3)
                        nc.vector.tensor_scalar_add(oc[:], ps_oT[hl][:], 0.0)
                        pend_norm.append((pair * 2 + hl, qc, denc, oc))
                    # ... then the previous row's reciprocal + normalize
                    while len(pend_norm) > 2:
                        _emit_norm(nc, rtmp, pend_norm.pop(0), oT_sb)
            while pend_norm:
                _emit_norm(nc, rtmp, pend_norm.pop(0), oT_sb)

            # ---- output projection: r[t, m] = sum_h oT_h.T @ wo_h ----
            ridx = 0
            for mc in range(4):
                wo_mc = wopool.tile([P, HPG, NQ], BF16, tag="womc",
                                    name="wo_mc")
                ld(wo_mc[:], wo_d[:, :, mc * NQ:(mc + 1) * NQ])
                for tb in range(TB):
                    ps_r = psum.tile([P, NQ], F32,
                                     tag=("b4", "b5", "b6", "b7")[ridx % 4],
                                     name="ps_r")
                    for h4 in range(HPG):
                        nc.tensor.matmul(
                            ps_r[:],
                            oT_sb[:, h4 * T + tb * P:h4 * T + (tb + 1) * P],
                            wo_mc[:, h4, :],
                            start=(h4 == 0), stop=(h4 == HPG - 1))
                    ro = rout.tile([P, NQ], BF16, tag="ro", name="ro")
                    nc.scalar.activation(ro[:], ps_r[:], AFT.Copy)
                    eng = nc.sync if ridx % 2 == 0 else nc.gpsimd
                    eng.dma_start(
                        out=r_d[tb * P:(tb + 1) * P, mc * NQ:(mc + 1) * NQ],
                        in_=ro[:])
                    ridx += 1

    nc.compile()
    return nc


def _emit_pv(nc, item, ps_oT, ps_den, v_sb, ones_sb):
    hl, j, q0, pT, st, sp = item
    nc.tensor.matmul(
        ps_oT[hl][:, q0:NQ],
        v_sb[:, j * 256 + hl * P:j * 256 + hl * P + P],
        pT[:, q0:NQ], start=st, stop=sp)
    nc.tensor.matmul(
        ps_den[hl][:, q0:NQ], ones_sb[:],
        pT[:, q0:NQ], start=st, stop=sp)


def _emit_norm(nc, rtmp, item, oT_sb):
    import concourse.mybir as mybir

    F32 = mybir.dt.float32
    h4, qc, denc, oc = item
    rec = rtmp.tile([P, NQ], F32, tag="rec", name="rec", bufs=2)
    nc.vector.reciprocal(rec[:], denc[:])
    nc.vector.tensor_mul(
        oT_sb[:, h4 * T + qc * NQ:h4 * T + (qc + 1) * NQ], oc[:], rec[:])


def _host_constants():
    import ml_dtypes

    BF = ml_dtypes.bfloat16
    half = D // 2
    pos = np.arange(T, dtype=np.float64)[:, None]
    freqs = np.power(
        np.float64(ROTARY_BASE),
        -np.arange(half, dtype=np.float64) / np.float64(half))[None, :]
    rad = pos * freqs                               # [T, 64]
    cos = np.cos(rad).T                             # [64, T]
    tan = np.tan(rad).T                             # [64, T] = sin/cos
    cc = np.concatenate([cos, cos], axis=0).astype(BF)
    tt = np.concatenate([tan, tan], axis=0).astype(BF)

    kk = np.arange(P)[:, None]
    qq = np.arange(P)[None, :]
    tri = np.where(kk <= qq, 0.0, -1e30).astype(BF)  # [128, 128]
    ones = np.ones((P, P), dtype=BF)
    ident = np.eye(P, dtype=np.float32).astype(BF)
    return cc, tt, tri, ones, ident


def kernel(x, wq, wk, wv, wo):
    import ml_dtypes

    BF = ml_dtypes.bfloat16

    x = np.asarray(x, dtype=np.float32)
    wq = np.asarray(wq, dtype=np.float32)
    wk = np.asarray(wk, dtype=np.float32)
    wv = np.asarray(wv, dtype=np.float32)
    wo = np.asarray(wo, dtype=np.float32)

    from concourse.bass_utils import run_bass_kernel_spmd

    if "nc" not in _CACHE:
        _CACHE["nc"] = _build_program()
    nc = _CACHE["nc"]

    cc, tt, tri, ones, ident = _host_constants()
    mult = np.float32(np.sqrt(ATTN_SCALE))

    def w_layout(w, g, scale):
        # w: [M, H, D] -> group slice -> [P, 2, MB, 256] bf16
        ws = (w[:, g * HPG:(g + 1) * HPG, :] * scale).astype(np.float32)
        ws = ws.reshape(M, 2, 256)                    # pair-major head axis
        ws = ws.reshape(MB, P, 2, 256).transpose(1, 2, 0, 3)
        return np.ascontiguousarray(ws).astype(BF)

    xts = [np.ascontiguousarray(x[b].T).astype(BF) for b in range(B)]
    in_maps = []
    for c in range(N_CORES):
        b, g = divmod(c, GROUPS)
        wog = np.ascontiguousarray(
            wo[g * HPG:(g + 1) * HPG].transpose(1, 0, 2)).astype(BF)
        in_maps.append({
            "xt": xts[b],
            "wq": w_layout(wq, g, mult),
            "wk": w_layout(wk, g, mult),
            "wv": w_layout(wv, g, np.float32(1.0)),
            "wo": wog,
            "trig_cc": cc,
            "trig_tt": tt,
            "tri_neg": tri,
            "onesw": ones,
            "identw": ident,
        })

    res = run_bass_kernel_spmd(nc, in_maps, list(range(N_CORES)))

    r = np.zeros((B, T, M), dtype=np.float32)
    for c in range(N_CORES):
        b = c // GROUPS
        r[b] += np.asarray(res.results[c]["r_out"], dtype=np.float32)
    return r


# revision 19
# speedup vs baseline: 1.5962x; 1.0820x over previous
"""Multi-head causal attention (RoPE, muP scale) on 8 TRN2 NeuronCores.

Sharding: core c = (b, g) with b = c // 4 (data-parallel batch), g = c % 4
(tensor-parallel head group of 4 heads).  Each core computes q/k/v
projections for its 4 heads, RoPE, causal flash-style attention in the
transposed (sT = [k, q]) orientation, and a partial output projection
o @ wo over its heads.  The host sums the 4 per-group partials per batch
(the tensor-parallel reduce) and stacks the 2 batches.

All matmul operands are bf16 (PSUM accumulation stays fp32): x is held
resident in SBUF (8 MB) and loaded once via per-mb-block DMAs so the
first projection matmul issues ~2us in.  The causal mask is applied on
the tensor engine as an identity-matmul accumulate of a -1e30 upper
triangle into the diagonal score blocks before exp; softmax denominators
come from an all-ones matmul.  RoPE uses a tan formulation so a single
DVE multiply retires each projection PSUM bank.  PSUM banks are assigned
explicitly (b0..b7: q/k accumulation on b0..b3, v and the attention
score/output-projection rotation on b4..b7); attention interleaves both
heads per j-block, runs pv/den two blocks behind exp so the PE never
waits on ACT, and drains oT/den to SBUF with cheap DVE copies so the
(slow) reciprocal runs entirely off the bank-reuse critical path.
"""

import sys

if "/opt/trn_rl_repo" not in sys.path:
    sys.path.insert(0, "/opt/trn_rl_repo")

import numpy as np

B, T, M, H, D = 2, 2048, 2048, 16, 128
N_CORES = 8
GROUPS = 4
HPG = H // GROUPS          # heads per group (4)
ROTARY_BASE = 10000.0
ATTN_SCALE = 1.0 / 128.0

P = 128                    # partitions
TC = T // 512              # 4 t-chunks of 512
MB = M // P                # 16 m-blocks
TB = T // P                # 16 t-blocks
NQ = 512                   # q-chunk width

_CACHE = {}


def _build_program():
    from concourse import bacc, tile
    import concourse.mybir as mybir

    F32 = mybir.dt.float32
    BF16 = mybir.dt.bfloat16
    AFT = mybir.ActivationFunctionType

    nc = bacc.Bacc("TRN2", target_bir_lowering=False, debug=False,
                   num_devices=N_CORES)

    xt_d = nc.dram_tensor("xt", [M, T], BF16, kind="ExternalInput")
    wq_d = nc.dram_tensor("wq", [P, 2, MB, 256], BF16, kind="ExternalInput")
    wk_d = nc.dram_tensor("wk", [P, 2, MB, 256], BF16, kind="ExternalInput")
    wv_d = nc.dram_tensor("wv", [P, 2, MB, 256], BF16, kind="ExternalInput")
    wo_d = nc.dram_tensor("wo", [P, HPG, M], BF16, kind="ExternalInput")
    cc_d = nc.dram_tensor("trig_cc", [P, T], BF16, kind="ExternalInput")
    tt_d = nc.dram_tensor("trig_tt", [P, T], BF16, kind="ExternalInput")
    tri_d = nc.dram_tensor("tri_neg", [P, P], BF16, kind="ExternalInput")
    ones_d = nc.dram_tensor("onesw", [P, P], BF16, kind="ExternalInput")
    id_d = nc.dram_tensor("identw", [P, P], BF16, kind="ExternalInput")
    r_d = nc.dram_tensor("r_out", [T, M], BF16, kind="ExternalOutput")

    with tile.TileContext(nc) as tc:
        with (
            tc.tile_pool(name="consts", bufs=1) as consts,
            tc.tile_pool(name="xpool", bufs=1) as xpool,
            tc.tile_pool(name="wpool", bufs=1) as wpool,
            tc.tile_pool(name="wopool", bufs=2) as wopool,
            tc.tile_pool(name="qkv", bufs=1) as qkv,
            tc.tile_pool(name="ppool", bufs=6) as ppool,
            tc.tile_pool(name="rtmp", bufs=2) as rtmp,
            tc.tile_pool(name="opool", bufs=1) as opool,
            tc.tile_pool(name="rout", bufs=6) as rout,
            tc.tile_pool(name="psum", bufs=1, space="PSUM") as psum,
        ):
            # --- load queues: alternate the two cheap DMA triggers ---
            qs = [nc.sync, nc.gpsimd]
            qi = [0]

            def ld(out, in_, q=None):
                eng = qs[qi[0] % 2] if q is None else q
                eng.dma_start(out=out, in_=in_)
                if q is None:
                    qi[0] += 1

            tri_sb = consts.tile([P, P], BF16, tag="tri")
            ld(tri_sb[:], tri_d[:])
            ones_sb = consts.tile([P, P], BF16, tag="ones")
            ld(ones_sb[:], ones_d[:])
            id_sb = consts.tile([P, P], BF16, tag="ident")
            ld(id_sb[:], id_d[:])

            # resident x^T [m, t], one DMA per 128-row m-block, interleaved
            # with the pair-0 weight quarters in consumption order
            xt_sb = xpool.tile([P, MB, T], BF16, tag="xt")
            wq_t = wpool.tile([P, MB, 256], BF16, tag="wq", name="wq_sb")
            wk_t = wpool.tile([P, MB, 256], BF16, tag="wk", name="wk_sb")
            wv_t = wpool.tile([P, MB, 256], BF16, tag="wv", name="wv_sb")

            def load_pair_weights(pair):
                q = None if pair == 0 else nc.sync
                for q4 in range(4):
                    sl = slice(4 * q4, 4 * q4 + 4)
                    if pair == 0:
                        # interleave in consumption order: x quarter-columns
                        # between the weight quarters (chunk-0 cols only;
                        # later chunks stream below)
                        mbs = list(range(4 * q4, 4 * q4 + 4))
                        ld(xt_sb[:, mbs[0], 0:NQ],
                           xt_d[mbs[0] * P:(mbs[0] + 1) * P, 0:NQ])
                        ld(wv_t[:, sl, :], wv_d[:, pair, sl, :], q=q)
                        ld(xt_sb[:, mbs[1], 0:NQ],
                           xt_d[mbs[1] * P:(mbs[1] + 1) * P, 0:NQ])
                        ld(wq_t[:, sl, :], wq_d[:, pair, sl, :], q=q)
                        ld(xt_sb[:, mbs[2], 0:NQ],
                           xt_d[mbs[2] * P:(mbs[2] + 1) * P, 0:NQ])
                        ld(wk_t[:, sl, :], wk_d[:, pair, sl, :], q=q)
                        ld(xt_sb[:, mbs[3], 0:NQ],
                           xt_d[mbs[3] * P:(mbs[3] + 1) * P, 0:NQ])
                    else:
                        ld(wv_t[:, sl, :], wv_d[:, pair, sl, :], q=q)
                        ld(wq_t[:, sl, :], wq_d[:, pair, sl, :], q=q)
                        ld(wk_t[:, sl, :], wk_d[:, pair, sl, :], q=q)

            load_pair_weights(0)
            cc_sb = consts.tile([P, T], BF16, tag="cc")
            ld(cc_sb[:], cc_d[:])
            tt_sb = consts.tile([P, T], BF16, tag="tt")
            ld(tt_sb[:], tt_d[:])
            for tcx in range(1, TC):
                for mb in range(MB):
                    ld(xt_sb[:, mb, tcx * NQ:(tcx + 1) * NQ],
                       xt_d[mb * P:(mb + 1) * P, tcx * NQ:(tcx + 1) * NQ])

            # oT for all 4 heads of the group: [d, h4 * T + t], bf16
            oT_sb = opool.tile([P, HPG * T], BF16, tag="oT")
            pend_norm = []   # deferred (h4, qc, denc, oc)

            # warmup: ramp the PE clock from ~0.5us on a memset tile
            warm_sb = consts.tile([P, NQ], BF16, tag="warm")
            nc.vector.memset(warm_sb[:], 0.0)
            ps_w = psum.tile([P, 2, NQ], F32, tag="B67", name="ps_warm")
            for wi in range(10):
                nc.tensor.matmul(ps_w[:, 1, :], warm_sb[0:P, 0:P],
                                 warm_sb[:], start=True, stop=True)

            wo_mcs = []
            for pair in range(2):
                deferred_rope = None
                qT_sb = [qkv.tile([P, T], BF16, tag=f"qT{hl}", name=f"qT{hl}")
                         for hl in range(2)]
                kT_sb = [qkv.tile([P, T], BF16, tag=f"kT{hl}", name=f"kT{hl}")
                         for hl in range(2)]
                v_sb = qkv.tile([P, TB * 256], BF16, tag="v", name="v_sb")

                # ---- projections + RoPE, one 512-wide t-chunk at a time ----
                for tcx in range(TC):
                    tsl = slice(tcx * NQ, (tcx + 1) * NQ)
                    B01 = psum.tile([P, 2, NQ], F32, tag="B01", name="B01")
                    B23 = psum.tile([P, 2, NQ], F32, tag="B23", name="B23")
                    B45 = psum.tile([P, 2, NQ], F32, tag="B45", name="B45")
                    B67 = psum.tile([P, 2, NQ], F32, tag="B67", name="B67")
                    psq = [B01[:, hl, :] for hl in range(2)]
                    psk = [B23[:, hl, :] for hl in range(2)]
                    # one full PSUM bank per v accumulation group (only the
                    # first 256 columns are written — a bank holds a single
                    # accumulation group)
                    psv = [B45[:, 0, :], B45[:, 1, :],
                           B67[:, 0, :], B67[:, 1, :]]

                    # emit v matmuls two m-blocks ahead of q/k so the chunk
                    # opens with work whose banks freed earliest
                    jobs = []
                    for mb in range(MB):
                        jobs.append(("v", mb))
                        if mb >= 4:
                            jobs.append(("qk", mb - 4))
                    jobs += [("qk", mb) for mb in range(MB - 4, MB)]

                    for kind, mb in jobs:
                        st, sp = (mb == 0), (mb == MB - 1)
                        if kind == "v":
                            for ts in range(4):
                                nc.tensor.matmul(
                                    psv[ts][:, 0:256],
                                    xt_sb[:, mb, tcx * NQ + ts * P:
                                          tcx * NQ + (ts + 1) * P],
                                    wv_t[:, mb, :], start=st, stop=sp)
                        else:
                            for hl in range(2):
                                nc.tensor.matmul(
                                    psq[hl],
                                    wq_t[:, mb, hl * P:(hl + 1) * P],
                                    xt_sb[:, mb, tsl], start=st, stop=sp)
                                nc.tensor.matmul(
                                    psk[hl],
                                    wk_t[:, mb, hl * P:(hl + 1) * P],
                                    xt_sb[:, mb, tsl], start=st, stop=sp)

                    # v bank drains (ACT) — free b4..b7 for the next chunk
                    hp = tc.high_priority()
                    hp.__enter__()
                    for ts in range(4):
                        tb = tcx * 4 + ts
                        nc.scalar.activation(
                            v_sb[:, tb * 256:(tb + 1) * 256],
                            psv[ts][:, 0:256], AFT.Copy)

                    # RoPE.  rot_even = qe*cos - qo*sin ; rot_odd = qe*sin +
                    # qo*cos.  pab = [qe*cos ; qo*cos] in one DVE op against
                    # the duplicated-cos table — the only reader of the
                    # projection PSUM bank (emitted q0,k0,q1,k1 to match the
                    # next chunk's bank-need order).  sh = swap(pab) (ACT),
                    # then the sin products are sh * tan.
                    paks = []
                    for hl in range(2):   # ACT bf16 copies retire k banks
                        a_k = rtmp.tile([P, NQ], BF16, tag="ak", name="a_k",
                                        bufs=2)
                        nc.scalar.activation(a_k[:], psk[hl], AFT.Copy)
                        paks.append(a_k)
                    phase2 = []
                    for hl in range(2):   # DVE muls retire q banks
                        pab = rtmp.tile([P, NQ], F32, tag="pab", name="pab",
                                        bufs=3)
                        nc.vector.tensor_mul(pab[:], psq[hl], cc_sb[:, tsl])
                        phase2.append((pab, qT_sb[hl]))
                    hp.__exit__(None, None, None)
                    # For the last chunk only the PSUM-reading ops (above)
                    # run here; the sh/pcd/combine tail feeds only attention
                    # row qc=3, so it is deferred into the attention loop to
                    # keep the first exps at the head of the ACT stream.
                    if tcx < TC - 1:
                        for hl in range(2):
                            pabk = rtmp.tile([P, NQ], BF16, tag="pabk",
                                             name="pabk", bufs=2)
                            nc.vector.tensor_mul(pabk[:], paks[hl][:],
                                                 cc_sb[:, tsl])
                            phase2.append((pabk, kT_sb[hl]))
                        _emit_rope_tail(nc, rtmp, tt_sb, tsl, phase2)
                    else:
                        deferred_rope = (paks, phase2, tsl)

                    # spread leftover deferred normalizations (previous
                    # pair's last rows) over the projection chunks
                    if pend_norm:
                        _emit_norm(nc, rtmp, pend_norm.pop(0), oT_sb)

                    if pair == 0 and tcx == TC - 1:
                        load_pair_weights(1)
                        for mc in range(4):
                            wo_mc = wopool.tile([P, HPG, NQ], BF16,
                                                tag="womc", name="wo_mc",
                                                bufs=4)
                            ld(wo_mc[:], wo_d[:, :, mc * NQ:(mc + 1) * NQ],
                               q=nc.sync)
                            wo_mcs.append(wo_mc)

                # ---- attention: both heads share a paired-bank score
                # tile so one wide exp covers them; pv/den run a few blocks
                # behind exp; oT/den drain to SBUF so the slow reciprocal
                # stays off the bank-reuse critical path ----
                sT_tags = ("B45", "B67")
                sidx = 0
                for qc in range(TC):
                    BA = psum.tile([P, 2, NQ], F32, tag="B01", name="BA")
                    BB = psum.tile([P, 2, NQ], F32, tag="B23", name="BB")
                    ps_oT = [BA[:, 0, :], BB[:, 0, :]]
                    ps_den = [BA[:, 1, :], BB[:, 1, :]]
                    jmax = 4 * qc + 3
                    pend_pv = []
                    for j in range(jmax + 1):
                        pat = j - 4 * qc
                        q0 = 128 * pat if pat >= 0 else 0
                        qs_ = slice(qc * NQ + q0, (qc + 1) * NQ)
                        st, sp = (j == 0), (j == jmax)
                        sT2 = psum.tile([P, 2, NQ], F32,
                                        tag=sT_tags[sidx % 2], name="sT2")
                        sidx += 1
                        for hl in range(2):
                            if pat >= 0:
                                nc.tensor.matmul(
                                    sT2[:, hl, q0:NQ],
                                    kT_sb[hl][:, j * P:(j + 1) * P],
                                    qT_sb[hl][:, qs_], start=True, stop=False)
                                nc.tensor.matmul(
                                    sT2[:, hl, q0:q0 + 128], id_sb[:],
                                    tri_sb[:], start=False, stop=True)
                            else:
                                nc.tensor.matmul(
                                    sT2[:, hl, q0:NQ],
                                    kT_sb[hl][:, j * P:(j + 1) * P],
                                    qT_sb[hl][:, qs_], start=True, stop=True)
                        pT2 = ppool.tile([P, 2, NQ], BF16, tag="pT",
                                         name="pT2")
                        nc.scalar.activation(
                            pT2[:, :, q0:NQ], sT2[:, :, q0:NQ], AFT.Exp)
                        pend_pv.append((j, q0, pT2, st, sp))
                        if len(pend_pv) > 2:
                            _emit_pv(nc, pend_pv.pop(0), ps_oT, ps_den,
                                     v_sb, ones_sb)
                    while pend_pv:
                        _emit_pv(nc, pend_pv.pop(0), ps_oT, ps_den, v_sb,
                                 ones_sb)
                    # drain copies (cheap, frees banks for qc+1) in the
                    # order the next accumulations need the banks back
                    ocs, dencs = [], []
                    for hl in range(2):
                        oc = rtmp.tile([P, NQ], BF16, tag="oc", name="oc",
                                       bufs=3)
                        nc.vector.tensor_scalar_add(oc[:], ps_oT[hl], 0.0)
                        ocs.append(oc)
                    for hl in range(2):
                        denc = rtmp.tile([P, NQ], F32, tag="denc",
                                         name="denc", bufs=3)
                        nc.vector.tensor_scalar_add(denc[:], ps_den[hl], 0.0)
                        dencs.append(denc)
                    for hl in range(2):
                        pend_norm.append((pair * 2 + hl, qc, dencs[hl],
                                          ocs[hl]))
                    if qc == 0 and deferred_rope is not None:
                        paks, ph2, tsl_ = deferred_rope
                        _emit_rope_tail(nc, rtmp, tt_sb, tsl_, ph2)
                    elif qc == 1 and deferred_rope is not None:
                        paks, ph2, tsl_ = deferred_rope
                        deferred_rope = None
                        ph2 = []
                        for hl in range(2):
                            pabk = rtmp.tile([P, NQ], BF16, tag="pabk",
                                             name="pabk", bufs=2)
                            nc.vector.tensor_mul(pabk[:], paks[hl][:],
                                                 cc_sb[:, tsl_])
                            ph2.append((pabk, kT_sb[hl]))
                        _emit_rope_tail(nc, rtmp, tt_sb, tsl_, ph2)
                    # ... then the previous row's reciprocal + normalize
                    while len(pend_norm) > 2:
                        _emit_norm(nc, rtmp, pend_norm.pop(0), oT_sb)
            while pend_norm:
                _emit_norm(nc, rtmp, pend_norm.pop(0), oT_sb)

            # ---- output projection: r[t, m] = sum_h oT_h.T @ wo_h ----
            # tb outer so the final deferred normalizations (qc=3 rows,
            # tb 12..15) have 3/4 of the phase as cover
            ridx = 0
            obanks = []
            for tb in range(TB):
                for mc in range(4):
                    wo_mc = wo_mcs[mc]
                    if ridx % 8 == 0:
                        obanks = [psum.tile([P, 2, NQ], F32, tag=t, name="pr")
                                  for t in ("B01", "B23", "B45", "B67")]
                    ps_r = obanks[ridx % 4][:, (ridx // 4) % 2, :]
                    for h4 in range(HPG):
                        nc.tensor.matmul(
                            ps_r,
                            oT_sb[:, h4 * T + tb * P:h4 * T + (tb + 1) * P],
                            wo_mc[:, h4, :],
                            start=(h4 == 0), stop=(h4 == HPG - 1))
                    ro = rout.tile([P, NQ], BF16, tag="ro", name="ro")
                    nc.scalar.activation(ro[:], ps_r, AFT.Copy)
                    eng = nc.sync if ridx % 2 == 0 else nc.gpsimd
                    eng.dma_start(
                        out=r_d[tb * P:(tb + 1) * P, mc * NQ:(mc + 1) * NQ],
                        in_=ro[:])
                    ridx += 1

    nc.compile()
    return nc


def _emit_rope_tail(nc, rtmp, tt_sb, tsl, phase2):
    import concourse.mybir as mybir

    BF16 = mybir.dt.bfloat16
    AFT = mybir.ActivationFunctionType
    for pab, dst in phase2:
        sh = rtmp.tile([P, NQ], BF16, tag="sh", name="sh", bufs=2)
        nc.scalar.activation(sh[0:64, :], pab[64:128, :], AFT.Copy)
        nc.scalar.activation(sh[64:128, :], pab[0:64, :], AFT.Copy)
        pcd = rtmp.tile([P, NQ], BF16, tag="pcd", name="pcd", bufs=2)
        nc.vector.tensor_mul(pcd[:], sh[:], tt_sb[:, tsl])
        nc.vector.tensor_sub(dst[0:64, tsl], pab[0:64, :], pcd[0:64, :])
        nc.vector.tensor_add(dst[64:128, tsl], pcd[64:128, :],
                             pab[64:128, :])


def _emit_pv(nc, item, ps_oT, ps_den, v_sb, ones_sb):
    j, q0, pT2, st, sp = item
    for hl in range(2):
        nc.tensor.matmul(
            ps_oT[hl][:, q0:NQ],
            v_sb[:, j * 256 + hl * P:j * 256 + hl * P + P],
            pT2[:, hl, q0:NQ], start=st, stop=sp)
        nc.tensor.matmul(
            ps_den[hl][:, q0:NQ], ones_sb[:],
            pT2[:, hl, q0:NQ], start=st, stop=sp)


def _emit_norm(nc, rtmp, item, oT_sb):
    import concourse.mybir as mybir

    F32 = mybir.dt.float32
    h4, qc, denc, oc = item
    rec = rtmp.tile([P, NQ], F32, tag="rec", name="rec", bufs=2)
    nc.vector.reciprocal(rec[:], denc[:])
    nc.gpsimd.tensor_mul(
        oT_sb[:, h4 * T + qc * NQ:h4 * T + (qc + 1) * NQ], oc[:], rec[:])


def _host_constants():
    import ml_dtypes

    BF = ml_dtypes.bfloat16
    half = D // 2
    pos = np.arange(T, dtype=np.float64)[:, None]
    freqs = np.power(
        np.float64(ROTARY_BASE),
        -np.arange(half, dtype=np.float64) / np.float64(half))[None, :]
    rad = pos * freqs                               # [T, 64]
    cos = np.cos(rad).T                             # [64, T]
    tan = np.tan(rad).T                             # [64, T] = sin/cos
    cc = np.concatenate([cos, cos], axis=0).astype(BF)
    tt = np.concatenate([tan, tan], axis=0).astype(BF)

    kk = np.arange(P)[:, None]
    qq = np.arange(P)[None, :]
    tri = np.where(kk <= qq, 0.0, -1e30).astype(BF)  # [128, 128]
    ones = np.ones((P, P), dtype=BF)
    ident = np.eye(P, dtype=np.float32).astype(BF)
    return cc, tt, tri, ones, ident


def kernel(x, wq, wk, wv, wo):
    import ml_dtypes

    BF = ml_dtypes.bfloat16

    x = np.asarray(x, dtype=np.float32)
    wq = np.asarray(wq, dtype=np.float32)
    wk = np.asarray(wk, dtype=np.float32)
    wv = np.asarray(wv, dtype=np.float32)
    wo = np.asarray(wo, dtype=np.float32)

    from concourse.bass_utils import run_bass_kernel_spmd

    if "nc" not in _CACHE:
        _CACHE["nc"] = _build_program()
    nc = _CACHE["nc"]

    cc, tt, tri, ones, ident = _host_constants()
    mult = np.float32(np.sqrt(ATTN_SCALE))

    def w_layout(w, g, scale):
        # w: [M, H, D] -> group slice -> [P, 2, MB, 256] bf16
        ws = (w[:, g * HPG:(g + 1) * HPG, :] * scale).astype(np.float32)
        ws = ws.reshape(M, 2, 256)                    # pair-major head axis
        ws = ws.reshape(MB, P, 2, 256).transpose(1, 2, 0, 3)
        return np.ascontiguousarray(ws).astype(BF)

    xts = [np.ascontiguousarray(x[b].T).astype(BF) for b in range(B)]
    in_maps = []
    for c in range(N_CORES):
        b, g = divmod(c, GROUPS)
        wog = np.ascontiguousarray(
            wo[g * HPG:(g + 1) * HPG].transpose(1, 0, 2)).astype(BF)
        in_maps.append({
            "xt": xts[b],
            "wq": w_layout(wq, g, mult),
            "wk": w_layout(wk, g, mult),
            "wv": w_layout(wv, g, np.float32(1.0)),
            "wo": wog,
            "trig_cc": cc,
            "trig_tt": tt,
            "tri_neg": tri,
            "onesw": ones,
            "identw": ident,
        })

    res = run_bass_kernel_spmd(nc, in_maps, list(range(N_CORES)))

    r = np.zeros((B, T, M), dtype=np.float32)
    for c in range(N_CORES):
        b = c // GROUPS
        r[b] += np.asarray(res.results[c]["r_out"], dtype=np.float32)
    return r


# revision 20
# speedup vs baseline: 1.6158x; 1.0123x over previous
"""Multi-head causal attention (RoPE, muP scale) on 8 TRN2 NeuronCores.

Sharding: core c = (b, g) with b = c // 4 (data-parallel batch), g = c % 4
(tensor-parallel head group of 4 heads).  Each core computes q/k/v
projections for its 4 heads, RoPE, causal flash-style attention in the
transposed (sT = [k, q]) orientation, and a partial output projection
o @ wo over its heads.  The host sums the 4 per-group partials per batch
(the tensor-parallel reduce) and stacks the 2 batches.

All matmul operands are bf16 (PSUM accumulation stays fp32): x is held
resident in SBUF (8 MB) and loaded once via per-mb-block DMAs so the
first projection matmul issues ~2us in.  The causal mask is applied on
the tensor engine as an identity-matmul accumulate of a -1e30 upper
triangle into the diagonal score blocks before exp; softmax denominators
come from an all-ones matmul.  RoPE uses a tan formulation so a single
DVE multiply retires each projection PSUM bank.  PSUM banks are assigned
explicitly (b0..b7: q/k accumulation on b0..b3, v and the attention
score/output-projection rotation on b4..b7); attention interleaves both
heads per j-block, runs pv/den two blocks behind exp so the PE never
waits on ACT, and drains oT/den to SBUF with cheap DVE copies so the
(slow) reciprocal runs entirely off the bank-reuse critical path.
"""

import sys

if "/opt/trn_rl_repo" not in sys.path:
    sys.path.insert(0, "/opt/trn_rl_repo")

import numpy as np

B, T, M, H, D = 2, 2048, 2048, 16, 128
N_CORES = 8
GROUPS = 4
HPG = H // GROUPS          # heads per group (4)
ROTARY_BASE = 10000.0
ATTN_SCALE = 1.0 / 128.0

P = 128                    # partitions
TC = T // 512              # 4 t-chunks of 512
MB = M // P                # 16 m-blocks
TB = T // P                # 16 t-blocks
NQ = 512                   # q-chunk width

_CACHE = {}


def _build_program():
    from concourse import bacc, tile
    import concourse.mybir as mybir

    F32 = mybir.dt.float32
    BF16 = mybir.dt.bfloat16
    AFT = mybir.ActivationFunctionType

    nc = bacc.Bacc("TRN2", target_bir_lowering=False, debug=False,
                   num_devices=N_CORES)

    xt_d = nc.dram_tensor("xt", [M, T], BF16, kind="ExternalInput")
    wq_d = nc.dram_tensor("wq", [P, 2, MB, 256], BF16, kind="ExternalInput")
    wk_d = nc.dram_tensor("wk", [P, 2, MB, 256], BF16, kind="ExternalInput")
    wv_d = nc.dram_tensor("wv", [P, 2, MB, 256], BF16, kind="ExternalInput")
    wo_d = nc.dram_tensor("wo", [P, HPG, M], BF16, kind="ExternalInput")
    cc_d = nc.dram_tensor("trig_cc", [P, T], BF16, kind="ExternalInput")
    tt_d = nc.dram_tensor("trig_tt", [P, T], BF16, kind="ExternalInput")
    tri_d = nc.dram_tensor("tri_neg", [P, P], BF16, kind="ExternalInput")
    ones_d = nc.dram_tensor("onesw", [P, P], BF16, kind="ExternalInput")
    id_d = nc.dram_tensor("identw", [P, P], BF16, kind="ExternalInput")
    r_d = nc.dram_tensor("r_out", [T, M], BF16, kind="ExternalOutput")

    with tile.TileContext(nc) as tc:
        with (
            tc.tile_pool(name="consts", bufs=1) as consts,
            tc.tile_pool(name="xpool", bufs=1) as xpool,
            tc.tile_pool(name="wpool", bufs=1) as wpool,
            tc.tile_pool(name="wopool", bufs=2) as wopool,
            tc.tile_pool(name="qkv", bufs=1) as qkv,
            tc.tile_pool(name="ppool", bufs=6) as ppool,
            tc.tile_pool(name="rtmp", bufs=2) as rtmp,
            tc.tile_pool(name="opool", bufs=1) as opool,
            tc.tile_pool(name="rout", bufs=6) as rout,
            tc.tile_pool(name="psum", bufs=1, space="PSUM") as psum,
        ):
            # --- load queues: alternate the two cheap DMA triggers ---
            qs = [nc.sync, nc.gpsimd]
            qi = [0]

            def ld(out, in_, q=None):
                eng = qs[qi[0] % 2] if q is None else q
                eng.dma_start(out=out, in_=in_)
                if q is None:
                    qi[0] += 1

            tri_sb = consts.tile([P, P], BF16, tag="tri")
            ld(tri_sb[:], tri_d[:])
            ones_sb = consts.tile([P, P], BF16, tag="ones")
            ld(ones_sb[:], ones_d[:])
            id_sb = consts.tile([P, P], BF16, tag="ident")
            ld(id_sb[:], id_d[:])

            # resident x^T [m, t], one DMA per 128-row m-block, interleaved
            # with the pair-0 weight quarters in consumption order
            xt_sb = xpool.tile([P, MB, T], BF16, tag="xt")
            wq_t = wpool.tile([P, MB, 256], BF16, tag="wq", name="wq_sb")
            wk_t = wpool.tile([P, MB, 256], BF16, tag="wk", name="wk_sb")
            wv_t = wpool.tile([P, MB, 256], BF16, tag="wv", name="wv_sb")

            def load_pair_weights(pair):
                q = None if pair == 0 else nc.sync
                for q4 in range(4):
                    sl = slice(4 * q4, 4 * q4 + 4)
                    if pair == 0:
                        # interleave in consumption order: x quarter-columns
                        # between the weight quarters (chunk-0 cols only;
                        # later chunks stream below)
                        mbs = list(range(4 * q4, 4 * q4 + 4))
                        ld(xt_sb[:, mbs[0], 0:NQ],
                           xt_d[mbs[0] * P:(mbs[0] + 1) * P, 0:NQ])
                        ld(wv_t[:, sl, :], wv_d[:, pair, sl, :], q=q)
                        ld(xt_sb[:, mbs[1], 0:NQ],
                           xt_d[mbs[1] * P:(mbs[1] + 1) * P, 0:NQ])
                        ld(wq_t[:, sl, :], wq_d[:, pair, sl, :], q=q)
                        ld(xt_sb[:, mbs[2], 0:NQ],
                           xt_d[mbs[2] * P:(mbs[2] + 1) * P, 0:NQ])
                        ld(wk_t[:, sl, :], wk_d[:, pair, sl, :], q=q)
                        ld(xt_sb[:, mbs[3], 0:NQ],
                           xt_d[mbs[3] * P:(mbs[3] + 1) * P, 0:NQ])
                    else:
                        ld(wv_t[:, sl, :], wv_d[:, pair, sl, :], q=q)
                        ld(wq_t[:, sl, :], wq_d[:, pair, sl, :], q=q)
                        ld(wk_t[:, sl, :], wk_d[:, pair, sl, :], q=q)

            load_pair_weights(0)
            cc_sb = consts.tile([P, T], BF16, tag="cc")
            ld(cc_sb[:], cc_d[:])
            tt_sb = consts.tile([P, T], BF16, tag="tt")
            ld(tt_sb[:], tt_d[:])
            for tcx in range(1, TC):
                for mb in range(MB):
                    ld(xt_sb[:, mb, tcx * NQ:(tcx + 1) * NQ],
                       xt_d[mb * P:(mb + 1) * P, tcx * NQ:(tcx + 1) * NQ])

            # oT for all 4 heads of the group: [d, h4 * T + t], bf16
            oT_sb = opool.tile([P, HPG * T], BF16, tag="oT")
            pend_norm = []   # deferred (h4, qc, denc, oc)

            # warmup: ramp the PE clock from ~0.5us on a memset tile
            warm_sb = consts.tile([P, NQ], BF16, tag="warm")
            nc.vector.memset(warm_sb[:], 0.0)
            ps_w = psum.tile([P, 2, NQ], F32, tag="B67", name="ps_warm")
            for wi in range(10):
                nc.tensor.matmul(ps_w[:, 1, :], warm_sb[0:P, 0:P],
                                 warm_sb[:], start=True, stop=True)

            wo_mcs = []
            for pair in range(2):
                deferred_rope = None
                qT_sb = [qkv.tile([P, T], BF16, tag=f"qT{hl}", name=f"qT{hl}")
                         for hl in range(2)]
                kT_sb = [qkv.tile([P, T], BF16, tag=f"kT{hl}", name=f"kT{hl}")
                         for hl in range(2)]
                v_sb = qkv.tile([P, TB * 256], BF16, tag="v", name="v_sb")

                # ---- projections + RoPE, one 512-wide t-chunk at a time ----
                for tcx in range(TC):
                    tsl = slice(tcx * NQ, (tcx + 1) * NQ)
                    B01 = psum.tile([P, 2, NQ], F32, tag="B01", name="B01")
                    B23 = psum.tile([P, 2, NQ], F32, tag="B23", name="B23")
                    B45 = psum.tile([P, 2, NQ], F32, tag="B45", name="B45")
                    B67 = psum.tile([P, 2, NQ], F32, tag="B67", name="B67")
                    psq = [B01[:, hl, :] for hl in range(2)]
                    psk = [B23[:, hl, :] for hl in range(2)]
                    # one full PSUM bank per v accumulation group (only the
                    # first 256 columns are written — a bank holds a single
                    # accumulation group)
                    psv = [B45[:, 0, :], B45[:, 1, :],
                           B67[:, 0, :], B67[:, 1, :]]

                    # emit v matmuls two m-blocks ahead of q/k so the chunk
                    # opens with work whose banks freed earliest
                    jobs = []
                    for mb in range(MB):
                        jobs.append(("v", mb))
                        if mb >= 4:
                            jobs.append(("qk", mb - 4))
                    jobs += [("qk", mb) for mb in range(MB - 4, MB)]

                    for kind, mb in jobs:
                        st, sp = (mb == 0), (mb == MB - 1)
                        if kind == "v":
                            for ts in range(4):
                                nc.tensor.matmul(
                                    psv[ts][:, 0:256],
                                    xt_sb[:, mb, tcx * NQ + ts * P:
                                          tcx * NQ + (ts + 1) * P],
                                    wv_t[:, mb, :], start=st, stop=sp)
                        else:
                            for hl in range(2):
                                nc.tensor.matmul(
                                    psq[hl],
                                    wq_t[:, mb, hl * P:(hl + 1) * P],
                                    xt_sb[:, mb, tsl], start=st, stop=sp)
                                nc.tensor.matmul(
                                    psk[hl],
                                    wk_t[:, mb, hl * P:(hl + 1) * P],
                                    xt_sb[:, mb, tsl], start=st, stop=sp)

                    # v bank drains (ACT) — free b4..b7 for the next chunk
                    hp = tc.high_priority()
                    hp.__enter__()
                    for ts in range(4):
                        tb = tcx * 4 + ts
                        nc.scalar.activation(
                            v_sb[:, tb * 256:(tb + 1) * 256],
                            psv[ts][:, 0:256], AFT.Copy)

                    # RoPE.  rot_even = qe*cos - qo*sin ; rot_odd = qe*sin +
                    # qo*cos.  pab = [qe*cos ; qo*cos] in one DVE op against
                    # the duplicated-cos table — the only reader of the
                    # projection PSUM bank (emitted q0,k0,q1,k1 to match the
                    # next chunk's bank-need order).  sh = swap(pab) (ACT),
                    # then the sin products are sh * tan.
                    paks = []
                    for hl in range(2):   # ACT bf16 copies retire k banks
                        a_k = rtmp.tile([P, NQ], BF16, tag="ak", name="a_k",
                                        bufs=2)
                        nc.scalar.activation(a_k[:], psk[hl], AFT.Copy)
                        paks.append(a_k)
                    phase2 = []
                    for hl in range(2):   # DVE muls retire q banks
                        pab = rtmp.tile([P, NQ], F32, tag="pab", name="pab",
                                        bufs=3)
                        nc.vector.tensor_mul(pab[:], psq[hl], cc_sb[:, tsl])
                        phase2.append((pab, qT_sb[hl]))
                    hp.__exit__(None, None, None)
                    # For the last chunk only the PSUM-reading ops (above)
                    # run here; the sh/pcd/combine tail feeds only attention
                    # row qc=3, so it is deferred into the attention loop to
                    # keep the first exps at the head of the ACT stream.
                    if tcx < TC - 1:
                        for hl in range(2):
                            pabk = rtmp.tile([P, NQ], BF16, tag="pabk",
                                             name="pabk", bufs=2)
                            nc.vector.tensor_mul(pabk[:], paks[hl][:],
                                                 cc_sb[:, tsl])
                            phase2.append((pabk, kT_sb[hl]))
                        _emit_rope_tail(nc, rtmp, tt_sb, tsl, phase2)
                    else:
                        deferred_rope = (paks, phase2, tsl)

                    # spread leftover deferred normalizations (previous
                    # pair's last rows) over the projection chunks
                    if pend_norm:
                        _emit_norm(nc, rtmp, pend_norm.pop(0), oT_sb)

                    if pair == 0 and tcx == TC - 1:
                        load_pair_weights(1)
                        for mc in range(4):
                            wo_mc = wopool.tile([P, HPG, NQ], BF16,
                                                tag="womc", name="wo_mc",
                                                bufs=4)
                            ld(wo_mc[:], wo_d[:, :, mc * NQ:(mc + 1) * NQ],
                               q=nc.sync)
                            wo_mcs.append(wo_mc)

                # ---- attention: both heads share a paired-bank score
                # tile so one wide exp covers them; pv/den run a few blocks
                # behind exp; oT/den drain to SBUF so the slow reciprocal
                # stays off the bank-reuse critical path ----
                sT_tags = ("B45", "B67")
                sidx = 0
                for qc in range(TC):
                    BA = psum.tile([P, 2, NQ], F32, tag="B01", name="BA")
                    BB = psum.tile([P, 2, NQ], F32, tag="B23", name="BB")
                    ps_oT = [BA[:, 0, :], BB[:, 0, :]]
                    ps_den = [BA[:, 1, :], BB[:, 1, :]]
                    jmax = 4 * qc + 3
                    pend_pv = []
                    for j in range(jmax + 1):
                        pat = j - 4 * qc
                        q0 = 128 * pat if pat >= 0 else 0
                        qs_ = slice(qc * NQ + q0, (qc + 1) * NQ)
                        st, sp = (j == 0), (j == jmax)
                        sT2 = psum.tile([P, 2, NQ], F32,
                                        tag=sT_tags[sidx % 2], name="sT2")
                        sidx += 1
                        for hl in range(2):
                            if pat >= 0:
                                nc.tensor.matmul(
                                    sT2[:, hl, q0:NQ],
                                    kT_sb[hl][:, j * P:(j + 1) * P],
                                    qT_sb[hl][:, qs_], start=True, stop=False)
                                nc.tensor.matmul(
                                    sT2[:, hl, q0:q0 + 128], id_sb[:],
                                    tri_sb[:], start=False, stop=True)
                            else:
                                nc.tensor.matmul(
                                    sT2[:, hl, q0:NQ],
                                    kT_sb[hl][:, j * P:(j + 1) * P],
                                    qT_sb[hl][:, qs_], start=True, stop=True)
                        pT2 = ppool.tile([P, 2, NQ], BF16, tag="pT",
                                         name="pT2")
                        nc.scalar.activation(
                            pT2[:, :, q0:NQ], sT2[:, :, q0:NQ], AFT.Exp)
                        pend_pv.append((j, q0, pT2, st, sp))
                        if len(pend_pv) > 2:
                            _emit_pv(nc, pend_pv.pop(0), ps_oT, ps_den,
                                     v_sb, ones_sb)
                    while pend_pv:
                        _emit_pv(nc, pend_pv.pop(0), ps_oT, ps_den, v_sb,
                                 ones_sb)
                    # drain copies (cheap, frees banks for qc+1) in the
                    # order the next accumulations need the banks back
                    ocs, dencs = [], []
                    for hl in range(2):
                        oc = rtmp.tile([P, NQ], BF16, tag="oc", name="oc",
                                       bufs=3)
                        nc.vector.tensor_scalar_add(oc[:], ps_oT[hl], 0.0)
                        ocs.append(oc)
                    for hl in range(2):
                        denc = rtmp.tile([P, NQ], F32, tag="denc",
                                         name="denc", bufs=3)
                        nc.vector.tensor_scalar_add(denc[:], ps_den[hl], 0.0)
                        dencs.append(denc)
                    for hl in range(2):
                        pend_norm.append((pair * 2 + hl, qc, dencs[hl],
                                          ocs[hl]))
                    if qc == 1 and deferred_rope is not None:
                        paks, ph2, tsl_ = deferred_rope
                        _emit_rope_tail(nc, rtmp, tt_sb, tsl_, ph2)
                    elif qc == 2 and deferred_rope is not None:
                        paks, ph2, tsl_ = deferred_rope
                        deferred_rope = None
                        ph2 = []
                        for hl in range(2):
                            pabk = rtmp.tile([P, NQ], BF16, tag="pabk",
                                             name="pabk", bufs=2)
                            nc.vector.tensor_mul(pabk[:], paks[hl][:],
                                                 cc_sb[:, tsl_])
                            ph2.append((pabk, kT_sb[hl]))
                        _emit_rope_tail(nc, rtmp, tt_sb, tsl_, ph2)
                    # ... then the previous row's reciprocal + normalize
                    while len(pend_norm) > 2:
                        _emit_norm(nc, rtmp, pend_norm.pop(0), oT_sb)
            while pend_norm:
                _emit_norm(nc, rtmp, pend_norm.pop(0), oT_sb)

            # ---- output projection: r[t, m] = sum_h oT_h.T @ wo_h ----
            # tb outer so the final deferred normalizations (qc=3 rows,
            # tb 12..15) have 3/4 of the phase as cover
            ridx = 0
            obanks = []
            for tb in range(TB):
                for mc in range(4):
                    wo_mc = wo_mcs[mc]
                    if ridx % 8 == 0:
                        obanks = [psum.tile([P, 2, NQ], F32, tag=t, name="pr")
                                  for t in ("B45", "B67", "B01", "B23")]
                    ps_r = obanks[ridx % 4][:, (ridx // 4) % 2, :]
                    for h4 in range(HPG):
                        nc.tensor.matmul(
                            ps_r,
                            oT_sb[:, h4 * T + tb * P:h4 * T + (tb + 1) * P],
                            wo_mc[:, h4, :],
                            start=(h4 == 0), stop=(h4 == HPG - 1))
                    ro = rout.tile([P, NQ], BF16, tag="ro", name="ro")
                    nc.scalar.activation(ro[:], ps_r, AFT.Copy)
                    eng = nc.sync if (ridx % 2 == 0 or tb >= 12) \
                        else nc.gpsimd
                    eng.dma_start(
                        out=r_d[tb * P:(tb + 1) * P, mc * NQ:(mc + 1) * NQ],
                        in_=ro[:])
                    ridx += 1

    nc.compile()
    return nc


def _emit_rope_tail(nc, rtmp, tt_sb, tsl, phase2):
    import concourse.mybir as mybir

    BF16 = mybir.dt.bfloat16
    AFT = mybir.ActivationFunctionType
    for pab, dst in phase2:
        sh = rtmp.tile([P, NQ], BF16, tag="sh", name="sh", bufs=2)
        nc.scalar.activation(sh[0:64, :], pab[64:128, :], AFT.Copy)
        nc.scalar.activation(sh[64:128, :], pab[0:64, :], AFT.Copy)
        pcd = rtmp.tile([P, NQ], BF16, tag="pcd", name="pcd", bufs=2)
        nc.vector.tensor_mul(pcd[:], sh[:], tt_sb[:, tsl])
        nc.vector.tensor_sub(dst[0:64, tsl], pab[0:64, :], pcd[0:64, :])
        nc.vector.tensor_add(dst[64:128, tsl], pcd[64:128, :],
                             pab[64:128, :])


def _emit_pv(nc, item, ps_oT, ps_den, v_sb, ones_sb):
    j, q0, pT2, st, sp = item
    for hl in range(2):
        nc.tensor.matmul(
            ps_oT[hl][:, q0:NQ],
            v_sb[:, j * 256 + hl * P:j * 256 + hl * P + P],
            pT2[:, hl, q0:NQ], start=st, stop=sp)
        nc.tensor.matmul(
            ps_den[hl][:, q0:NQ], ones_sb[:],
            pT2[:, hl, q0:NQ], start=st, stop=sp)


def _emit_norm(nc, rtmp, item, oT_sb):
    import concourse.mybir as mybir

    F32 = mybir.dt.float32
    h4, qc, denc, oc = item
    rec = rtmp.tile([P, NQ], F32, tag="rec", name="rec", bufs=2)
    nc.vector.reciprocal(rec[:], denc[:])
    nc.gpsimd.tensor_mul(
        oT_sb[:, h4 * T + qc * NQ:h4 * T + (qc + 1) * NQ], oc[:], rec[:])


def _host_constants():
    import ml_dtypes

    BF = ml_dtypes.bfloat16
    half = D // 2
    pos = np.arange(T, dtype=np.float64)[:, None]
    freqs = np.power(
        np.float64(ROTARY_BASE),
        -np.arange(half, dtype=np.float64) / np.float64(half))[None, :]
    rad = pos * freqs                               # [T, 64]
    cos = np.cos(rad).T                             # [64, T]
    tan = np.tan(rad).T                             # [64, T] = sin/cos
    cc = np.concatenate([cos, cos], axis=0).astype(BF)
    tt = np.concatenate([tan, tan], axis=0).astype(BF)

    kk = np.arange(P)[:, None]
    qq = np.arange(P)[None, :]
    tri = np.where(kk <= qq, 0.0, -1e30).astype(BF)  # [128, 128]
    ones = np.ones((P, P), dtype=BF)
    ident = np.eye(P, dtype=np.float32).astype(BF)
    return cc, tt, tri, ones, ident


def kernel(x, wq, wk, wv, wo):
    import ml_dtypes

    BF = ml_dtypes.bfloat16

    x = np.asarray(x, dtype=np.float32)
    wq = np.asarray(wq, dtype=np.float32)
    wk = np.asarray(wk, dtype=np.float32)
    wv = np.asarray(wv, dtype=np.float32)
    wo = np.asarray(wo, dtype=np.float32)

    from concourse.bass_utils import run_bass_kernel_spmd

    if "nc" not in _CACHE:
        _CACHE["nc"] = _build_program()
    nc = _CACHE["nc"]

    cc, tt, tri, ones, ident = _host_constants()
    mult = np.float32(np.sqrt(ATTN_SCALE))

    def w_layout(w, g, scale):
        # w: [M, H, D] -> group slice -> [P, 2, MB, 256] bf16
        ws = (w[:, g * HPG:(g + 1) * HPG, :] * scale).astype(np.float32)
        ws = ws.reshape(M, 2, 256)                    # pair-major head axis
        ws = ws.reshape(MB, P, 2, 256).transpose(1, 2, 0, 3)
        return np.ascontiguousarray(ws).astype(BF)

    xts = [np.ascontiguousarray(x[b].T).astype(BF) for b in range(B)]
    in_maps = []
    for c in range(N_CORES):
        b, g = divmod(c, GROUPS)
        wog = np.ascontiguousarray(
            wo[g * HPG:(g + 1) * HPG].transpose(1, 0, 2)).astype(BF)
        in_maps.append({
            "xt": xts[b],
            "wq": w_layout(wq, g, mult),
            "wk": w_layout(wk, g, mult),
            "wv": w_layout(wv, g, np.float32(1.0)),
            "wo": wog,
            "trig_cc": cc,
            "trig_tt": tt,
            "tri_neg": tri,
            "onesw": ones,
            "identw": ident,
        })

    res = run_bass_kernel_spmd(nc, in_maps, list(range(N_CORES)))

    r = np.zeros((B, T, M), dtype=np.float32)
    for c in range(N_CORES):
        b = c // GROUPS
        r[b] += np.asarray(res.results[c]["r_out"], dtype=np.float32)
    return r
